# revision 1
# baseline (speedup 1.0000x reference)
"""Trainium2 Bass kernel for the CPC contrastive loss problem.

Math (reference):
    fx = relu(x @ W1 + b1) @ W2 + b2          [N, Z]
    fz = z @ Wz + bz                          [N, Z]
    u[n] = fx[n] @ Ws[c[n]]                   [N, Z]
    T = softplus(<u, fz>_row)                 [N]
    neg_T[i] = mean_{j: c[j]==c[i]} softplus(<u[i], fz[j]>)
    out = log(T + eps) - log(neg_T + eps)

Key optimization: neg_T[i] only involves same-category j's (~N/C = 128 of
8192), so rows are grouped by category on the host and S is computed in
per-category blocks instead of the full NxN matrix (64x less work).

Sharding: 64 categories -> 8 cores x 8 categories. Each category's rows are
zero-padded to a B=256 bucket (category sizes are Binomial(8192, 1/64);
max ~165 in practice; a numpy fallback covers overflow). The j-width of
each S block is trimmed to JW=192 since only n_k <= JW columns are real.
Padded fz columns are exactly zero (bias applied via a masked rank-1
matmul), so each padded column contributes exactly softplus(0)=ln2,
subtracted out with a host-precomputed correction.

Precision strategy:
  - MLP / u chain in float32r matmuls (tf32-like, ~1.5e-4): feeds both S
    and the diagonal dots d, whose error propagates ~1:1 into the output.
  - S-blocks in bf16 (~2.3e-3 on S): only enters through a 128-term
    softplus AVERAGE inside a log, contributing ~3e-4.
  - softplus(S) = relu(S) + log1p(exp(-|S|)) split: never under/overflows
    (sigmoid/softplus LUTs are unavailable or underflow at |S|~500).
  - The T term log(softplus(d) + 1e-8) needs ~1e-9 ABSOLUTE accuracy when
    d is very negative: computed on [128,16] via exp + branch-free log1p
    (series for e < 1e-4, Ln(1+e) otherwise).
"""

import sys

for _p in ("/opt/trn_rl_repo", "/root/.axon_site/_ro/trn_rl_repo"):
    if _p not in sys.path:
        sys.path.append(_p)

import numpy as np

import concourse.bacc as bacc
import concourse.tile as tile
from concourse import mybir as mb
from concourse.bass_utils import run_bass_kernel_spmd

# ---------------------------------------------------------------- constants
N, IN, Z, C, H = 8192, 512, 128, 64, 50
NCORES = 8
G = C // NCORES          # categories per core
B = 256                  # bucket (padded category) size, i-direction
JW = 192                 # j-width of S blocks (max category size <= JW)
R = G * B                # padded rows per core = 2048
NCHUNK = R // 128        # 16 row-chunks of 128
KX = IN // 128           # 4 k-tiles for x
NT = R // 512            # 4 n-tiles of 512
NH = 4                   # S-stage groups (PSUM-sized)
CPH = G // NH            # categories per group
EPS = 1e-8
LN2 = float(np.log(2.0))

# packed small-weight layout (columns of the [128, PW] packB tensor)
PK_W1 = (0, 200)         # [(k p) h -> p (k h)]
PK_BS = (200, 208)       # b2 @ Ws[g], one column per g
PK_WZ = (208, 336)
PK_SUBV = (336, 352)
PK_PINV = (352, 368)
PK_B1 = (368, 369)       # rows 0:50
PW = 369
PK_BZ = (0, 128)         # pack1 [1, PW1]
PK_MROW = (128, 128 + R)
PW1 = 128 + R

N_WARM = 8               # PE warm-up matmul count
BLOCKS = [(0, 2), (2, 2), (4, 2), (6, 1), (7, 1)]

F = mb.ActivationFunctionType
OP = mb.AluOpType
FP32 = mb.dt.float32
FP32R = mb.dt.float32r
BF16 = mb.dt.bfloat16

_PROGRAM = None


def _build_program():
    nc = bacc.Bacc("TRN2", target_bir_lowering=False, debug=False)

    d_xgT = nc.dram_tensor("xgT", [IN, R], FP32, kind="ExternalInput").ap()
    d_zgT = nc.dram_tensor("zgT", [Z, R], FP32, kind="ExternalInput").ap()
    # all small weights/vectors packed into two tensors -> two DMAs
    d_packB = nc.dram_tensor("packB", [128, PW], FP32, kind="ExternalInput").ap()
    d_pack1 = nc.dram_tensor("pack1", [1, PW1], FP32, kind="ExternalInput").ap()
    d_w2s = nc.dram_tensor("w2s", [H, G * Z], FP32, kind="ExternalInput").ap()
    d_yout = nc.dram_tensor("yout", [R], FP32, kind="ExternalOutput").ap()

    with tile.TileContext(nc) as tc:
        with (
            tc.tile_pool(name="const", bufs=1) as const,
            tc.tile_pool(name="junk", bufs=2) as junkp,
            tc.tile_pool(name="psum_mlp", bufs=3, space="PSUM") as psum_mlp,
            tc.tile_pool(name="psum_d", bufs=1, space="PSUM") as psum_dp,
            tc.tile_pool(name="psum_s", bufs=2, space="PSUM") as psum_sp,
        ):
            # ---- constants / packed weights
            s_ones = const.tile([128, 1], FP32)
            nc.vector.memset(s_ones[:], 1.0)
            s_eps = const.tile([128, 1], FP32)
            nc.vector.memset(s_eps[:], EPS)
            # Pre-load the ONE ACT table set containing every function this
            # kernel uses (Abs/Exp/Ln/Relu all live in
            # natural_log_exp_and_others, act_func_set_id 6). The automatic
            # chooser never picks it (first-match goes to exp_and_others or
            # natural_log), which would thrash mid-kernel.
            nc.scalar.add_instruction(
                mb.InstLoadActFuncSet(
                    name=nc.get_next_instruction_name(),
                    ins=[],
                    outs=[],
                    act_func_set_id=6,
                )
            )
            s_warmact = const.tile([128, 1], FP32)
            nc.scalar.activation(out=s_warmact[:], in_=s_ones[:], func=F.Abs)

            s_packB = const.tile([128, PW], FP32R)
            nc.sync.dma_start(out=s_packB[:], in_=d_packB.bitcast(FP32R)[:])
            s_pack1 = const.tile([1, PW1], FP32R)
            nc.sync.dma_start(out=s_pack1[:], in_=d_pack1.bitcast(FP32R)[:])

            def pk(lo_hi, rows=128, cast=None):
                ap = s_packB[0:rows, lo_hi[0] : lo_hi[1]]
                return ap.bitcast(cast) if cast else ap

            s_w1 = pk(PK_W1).rearrange("p (k h) -> p k h", k=KX)
            s_bs = pk(PK_BS, cast=FP32)
            s_wz = pk(PK_WZ)
            s_w2s = const.tile([H, G * Z], FP32R)
            nc.sync.dma_start(out=s_w2s[:], in_=d_w2s.bitcast(FP32R)[:])
            s_subv = pk(PK_SUBV, cast=FP32)
            s_pinv = pk(PK_PINV, cast=FP32)
            s_b1 = pk(PK_B1, rows=H, cast=FP32)
            s_bz = s_pack1[0:1, PK_BZ[0] : PK_BZ[1]]
            s_mrow = s_pack1[0:1, PK_MROW[0] : PK_MROW[1]]

            # PE warm-up: a few junk matmuls so HAM ramps during the DMA
            # (kept short: these occupy the in-order PE queue)
            pwarm = psum_mlp.tile([1, 64], FP32, tag="mlp")
            s_wrhs = const.tile([128, 64], FP32)
            nc.vector.memset(s_wrhs[:], 0.0)
            for _ in range(N_WARM):
                nc.tensor.matmul(
                    pwarm[:], lhsT=s_ones[:], rhs=s_wrhs[:], start=True, stop=True
                )

            # ---- persistent tiles
            s_zgT = const.tile([128, R], FP32R)
            s_xgT = const.tile([128, KX, R], FP32R)
            s_h1T = const.tile([H, R], FP32R)
            s_fzT = const.tile([128, R], FP32R)
            s_uT = const.tile([128, R], FP32R)
            s_u16 = const.tile([128, R], BF16)
            s_fz16 = const.tile([128, R], BF16)
            s_prod = const.tile([128, R], FP32)
            s_rel = const.tile([128, NCHUNK], FP32)
            s_d = const.tile([128, NCHUNK], FP32)
            # exp(-|.|): S chunks [0:16*JW], then the T-term diag [16*JW:]
            EALL = NCHUNK * JW + NCHUNK
            DOFF = NCHUNK * JW
            s_e_all = const.tile([128, EALL], FP32)
            s_lS = const.tile([128, DOFF], BF16)
            s_lT = const.tile([128, NCHUNK], FP32)
            s_lsum = const.tile([128, NCHUNK], FP32)
            s_q2c = const.tile([128, NCHUNK], FP32)

            x_view = d_xgT.bitcast(FP32R).rearrange("(k p) n -> p k n", p=128)
            pd = psum_dp.tile([128, NCHUNK], FP32)

            # ---- block-pipelined main loop. Early blocks are 2 categories
            # wide (fewer ops while DMA paces everything); the final block is
            # split into single categories to shorten the post-DMA tail chain.
            blocks = BLOCKS
            for g0, ncat in blocks:
                w = ncat * B
                ns = slice(g0 * B, g0 * B + w)
                # DMA: z slice then the 4 x k-chunks of this block
                nc.sync.dma_start(out=s_zgT[:, ns], in_=d_zgT.bitcast(FP32R)[:, ns])
                for k in range(KX):
                    nc.sync.dma_start(out=s_xgT[:, k, ns], in_=x_view[:, k, ns])

                # fz = Wz^T z + bz x mrow (rank-1 keeps padded columns zero)
                pz = psum_mlp.tile([128, w], FP32, tag="mlp")
                nc.tensor.matmul(
                    pz[:], lhsT=s_wz, rhs=s_zgT[:, ns], start=True, stop=False
                )
                nc.tensor.matmul(
                    pz[:], lhsT=s_bz, rhs=s_mrow[:, ns], start=False, stop=True
                )
                nc.vector.tensor_copy(s_fzT[:, ns], pz[:])
                nc.gpsimd.tensor_copy(s_fz16[:, ns], s_fzT.bitcast(FP32)[:, ns])

                # h1 = relu(W1^T x + b1)
                ph = psum_mlp.tile([H, w], FP32, tag="mlp")
                for k in range(KX):
                    nc.tensor.matmul(
                        ph[:],
                        lhsT=s_w1[:, k, :],
                        rhs=s_xgT[:, k, ns],
                        start=(k == 0),
                        stop=(k == KX - 1),
                    )
                nc.vector.tensor_scalar(
                    out=s_h1T[:, ns], in0=ph[:], scalar1=s_b1, scalar2=0.0,
                    op0=OP.add, op1=OP.max,
                )

                # per category: u = relu(h1) @ (W2 Ws[g]) + b2 Ws[g]
                # (W2@Ws folded on the host -> one fewer pipeline stage)
                pS = psum_sp.tile([128, ncat * 2, 256], FP32, tag="spsum")
                for gg in range(ncat):
                    g = g0 + gg
                    gs = slice(g * B, (g + 1) * B)
                    pu = psum_mlp.tile([128, B], FP32, tag="mlp")
                    nc.tensor.matmul(
                        pu[:], lhsT=s_w2s[:, g * Z : (g + 1) * Z],
                        rhs=s_h1T[:, gs], start=True, stop=True,
                    )
                    nc.vector.tensor_scalar_add(
                        s_uT[:, gs], pu[:], s_bs[:, g : g + 1]
                    )
                    nc.gpsimd.tensor_copy(s_u16[:, gs], s_uT.bitcast(FP32)[:, gs])
                    nc.gpsimd.tensor_mul(
                        s_prod[:, gs], s_uT.bitcast(FP32)[:, gs],
                        s_fzT.bitcast(FP32)[:, gs],
                    )
                    for h in range(2):
                        ci = 2 * g + h
                        # d chunk: prod^T @ ones
                        nc.tensor.matmul(
                            pd[:, ci : ci + 1],
                            lhsT=s_prod[:, ci * 128 : (ci + 1) * 128],
                            rhs=s_ones[:],
                            start=True,
                            stop=True,
                        )
                        # S chunk (bf16; PSUM stride 256 avoids bank splits)
                        nc.tensor.matmul(
                            pS[:, 2 * gg + h, :JW],
                            lhsT=s_u16[:, g * B + h * 128 : g * B + (h + 1) * 128],
                            rhs=s_fz16[:, gs.start : gs.start + JW],
                            start=True,
                            stop=True,
                        )
                        # relu row-sum straight from PSUM (alternating
                        # DVE tensor_scalar accum / ACT Relu accum)
                        jk = junkp.tile([128, JW], FP32, tag="junk")
                        if h == 0:
                            nc.vector.tensor_scalar(
                                out=jk[:], in0=pS[:, 2 * gg + h, :JW],
                                scalar1=0.0, scalar2=None, op0=OP.max, op1=OP.add,
                                accum_out=s_rel[:, ci : ci + 1],
                            )
                        else:
                            nc.scalar.activation(
                                out=jk[:], in_=pS[:, 2 * gg + h, :JW],
                                func=F.Relu,
                                accum_out=s_rel[:, ci : ci + 1],
                            )
                # |S|, exp(-|S|), log1p via Ln(e+1), and row-sum reduce:
                # all pipelined per block (single resident ACT table set)
                nch = ncat * 2
                es = slice(g0 * 2 * JW, (g0 * 2 + nch) * JW)
                s_a = junkp.tile([128, nch * JW], FP32, tag="abs")
                nc.scalar.activation(
                    out=s_a.rearrange("p (c j) -> p c j", j=JW),
                    in_=pS[:, :, :JW],
                    func=F.Abs,
                )
                nc.scalar.activation(
                    out=s_e_all[:, es], in_=s_a[:], func=F.Exp, scale=-1.0
                )
                nc.scalar.activation(
                    out=s_lS[:, es], in_=s_e_all[:, es], func=F.Ln, bias=1.0
                )
                nc.vector.tensor_reduce(
                    out=s_lsum[:, 2 * g0 : 2 * g0 + nch],
                    in_=s_lS[:, es].rearrange("p (c j) -> p c j", j=JW),
                    axis=mb.AxisListType.X,
                    op=OP.add,
                )
                # neg_T numerator per block: clamp((rel+lsum-subv)*pinv, 0)
                cs = slice(2 * g0, 2 * g0 + nch)
                jq = junkp.tile([128, nch], FP32, tag="jq")
                nc.vector.tensor_add(jq[:], s_rel[:, cs], s_lsum[:, cs])
                jq2 = junkp.tile([128, nch], FP32, tag="jq2")
                nc.vector.tensor_sub(jq2[:], jq[:], s_subv[:, cs])
                jq3 = junkp.tile([128, nch], FP32, tag="jq3")
                nc.vector.tensor_mul(jq3[:], jq2[:], s_pinv[:, cs])
                nc.vector.tensor_scalar_max(s_q2c[:, cs], jq3[:], 0.0)

            # ---- T-term diag d and its exp/log1p
            sm = const
            nc.vector.tensor_copy(s_d[:], pd[:])
            s_ad = const.tile([128, NCHUNK], FP32)
            nc.scalar.activation(out=s_ad[:], in_=s_d[:], func=F.Abs)
            nc.scalar.activation(
                out=s_e_all[:, DOFF:], in_=s_ad[:], func=F.Exp, scale=-1.0
            )
            nc.scalar.activation(
                out=s_lT[:], in_=s_e_all[:, DOFF:], func=F.Ln, bias=1.0
            )

            # T-term: softplus(d) = relu(d) + log1p(exp(-|d|)); small-e branch
            # in series form for absolute accuracy near the 1e-8 floor
            s_ed = s_e_all[:, DOFF:]
            s_t1 = sm.tile([128, NCHUNK], FP32)
            nc.vector.tensor_scalar(
                out=s_t1[:], in0=s_ed, scalar1=-0.5, scalar2=1.0,
                op0=OP.mult, op1=OP.add,
            )  # 1 - e/2
            s_ssm = sm.tile([128, NCHUNK], FP32)
            nc.vector.tensor_mul(s_ssm[:], s_ed, s_t1[:])  # e*(1 - e/2)
            s_mk = sm.tile([128, NCHUNK], mb.dt.int8)
            nc.vector.tensor_scalar(
                out=s_mk[:], in0=s_ed, scalar1=1e-4, scalar2=None, op0=OP.is_lt
            )
            s_l1p = sm.tile([128, NCHUNK], FP32)
            nc.vector.select(s_l1p[:], s_mk[:], s_ssm[:], s_lT[:])
            s_T0 = sm.tile([128, NCHUNK], FP32)
            nc.vector.scalar_tensor_tensor(
                out=s_T0[:], in0=s_d[:], scalar=0.0, in1=s_l1p[:],
                op0=OP.max, op1=OP.add,
            )  # relu(d) + log1p(exp(-|d|))
            s_logT = sm.tile([128, NCHUNK], FP32)
            nc.scalar.activation(out=s_logT[:], in_=s_T0[:], func=F.Ln, bias=s_eps[:])

            # ---- final output (neg_T numerators were computed per block)
            s_logn = sm.tile([128, NCHUNK], FP32)
            nc.scalar.activation(
                out=s_logn[:], in_=s_q2c[:], func=F.Ln, bias=s_eps[:]
            )
            s_y = sm.tile([128, NCHUNK], FP32)
            nc.vector.tensor_sub(s_y[:], s_logT[:], s_logn[:])

            nc.sync.dma_start(
                out=d_yout.rearrange("(c p) -> p c", p=128), in_=s_y[:]
            )

    nc.compile()
    return nc


def get_program():
    global _PROGRAM
    if _PROGRAM is None:
        _PROGRAM = _build_program()
    return _PROGRAM


# ---------------------------------------------------------------- host side
def _prep_core_inputs(x, z, Ws, W1, b1, W2, b2, Wz, bz, idx_lists, core):
    """Build the per-core input map (grouped, padded, transposed, packed)."""
    xgT = np.zeros((IN, R), np.float32)
    zgT = np.zeros((Z, R), np.float32)
    pack1 = np.zeros((1, PW1), np.float32)
    subv = np.zeros(R, np.float32)
    pinv = np.zeros(R, np.float32)
    for s in range(G):
        k = core * G + s
        idx = idx_lists[k]
        n = len(idx)
        lo = s * B
        if n:
            xgT[:, lo : lo + n] = x[idx].T
            zgT[:, lo : lo + n] = z[idx].T
            pack1[0, PK_MROW[0] + lo : PK_MROW[0] + lo + n] = 1.0
            subv[lo : lo + B] = (JW - n) * LN2
            pinv[lo : lo + B] = 1.0 / n
    pack1[0, PK_BZ[0] : PK_BZ[1]] = bz

    packB = np.zeros((128, PW), np.float32)
    packB[:, PK_W1[0] : PK_W1[1]] = (
        W1.reshape(KX, 128, H).transpose(1, 0, 2).reshape(128, KX * H)
    )
    packB[:, PK_WZ[0] : PK_WZ[1]] = Wz
    # chunk layout [128, NCHUNK]: row r = ci*128 + p  ->  [p, ci]
    packB[:, PK_SUBV[0] : PK_SUBV[1]] = subv.reshape(NCHUNK, 128).T
    packB[:, PK_PINV[0] : PK_PINV[1]] = pinv.reshape(NCHUNK, 128).T
    packB[:H, PK_B1[0]] = b1
    # fold the second MLP layer into each category's bilinear weight:
    # u = relu(h1) @ (W2 Ws[g]) + b2 Ws[g]
    Wsg64 = Ws[core * G : (core + 1) * G].astype(np.float64)
    w2s = np.ascontiguousarray(
        (W2.astype(np.float64) @ Wsg64).transpose(1, 0, 2).reshape(H, G * Z),
        dtype=np.float32,
    )
    bs = (b2.astype(np.float64) @ Wsg64).astype(np.float32)  # [G, Z]
    packB[:, PK_BS[0] : PK_BS[1]] = bs.T
    return {"xgT": xgT, "zgT": zgT, "packB": packB, "pack1": pack1, "w2s": w2s}


def _numpy_fallback(x, c, z, W1, b1, W2, b2, Wz, bz, Ws):
    x64 = x.astype(np.float64)
    fx = np.maximum(x64 @ W1.astype(np.float64) + b1, 0.0) @ W2.astype(
        np.float64
    ) + b2
    fz = z.astype(np.float64) @ Wz.astype(np.float64) + bz
    u = np.einsum("nd,nde->ne", fx, Ws.astype(np.float64)[c])

    def sp(v):
        return np.log1p(np.exp(-np.abs(v))) + np.maximum(v, 0.0)

    T = sp(np.einsum("ne,ne->n", u, fz))
    out = np.empty(N, np.float64)
    for k in range(C):
        idx = np.where(c == k)[0]
        if len(idx) == 0:
            continue
        Sk = sp(u[idx] @ fz[idx].T)
        neg = Sk.mean(axis=1)
        out[idx] = np.log(T[idx] + EPS) - np.log(neg + EPS)
    return out.astype(np.float32)


def kernel(x, c, z, W1, b1, W2, b2, Wz, bz, Ws):
    x = np.ascontiguousarray(np.asarray(x), dtype=np.float32)
    z = np.ascontiguousarray(np.asarray(z), dtype=np.float32)
    W1 = np.ascontiguousarray(np.asarray(W1), dtype=np.float32)
    b1 = np.ascontiguousarray(np.asarray(b1), dtype=np.float32)
    W2 = np.ascontiguousarray(np.asarray(W2), dtype=np.float32)
    b2 = np.ascontiguousarray(np.asarray(b2), dtype=np.float32)
    Wz = np.ascontiguousarray(np.asarray(Wz), dtype=np.float32)
    bz = np.ascontiguousarray(np.asarray(bz), dtype=np.float32)
    Ws = np.ascontiguousarray(np.asarray(Ws), dtype=np.float32)
    cf = np.asarray(c).reshape(-1).astype(np.int64)

    idx_lists = [np.where(cf == k)[0] for k in range(C)]
    if max(len(i) for i in idx_lists) > JW:
        return _numpy_fallback(x, cf, z, W1, b1, W2, b2, Wz, bz, Ws)

    in_maps = [
        _prep_core_inputs(x, z, Ws, W1, b1, W2, b2, Wz, bz, idx_lists, core)
        for core in range(NCORES)
    ]

    nc = get_program()
    res = run_bass_kernel_spmd(nc, in_maps, core_ids=list(range(NCORES)))

    out = np.empty(N, np.float32)
    for core in range(NCORES):
        y = res.results[core]["yout"]
        for s in range(G):
            k = core * G + s
            idx = idx_lists[k]
            if len(idx):
                out[idx] = y[s * B : s * B + len(idx)]
    return out



# revision 9
# speedup vs baseline: 1.3613x; 1.3613x over previous
"""Trainium2 Bass kernel for the CPC contrastive loss problem.

Math (reference):
    fx = relu(x @ W1 + b1) @ W2 + b2          [N, Z]
    fz = z @ Wz + bz                          [N, Z]
    u[n] = fx[n] @ Ws[c[n]]                   [N, Z]
    T = softplus(<u, fz>_row)                 [N]
    neg_T[i] = mean_{j: c[j]==c[i]} softplus(<u[i], fz[j]>)
    out = log(T + eps) - log(neg_T + eps)

Structure: rows are grouped by category on the host; each of the 8 cores gets
8 categories, so the NxN S matrix reduces to per-category blocks (64x less
work). Categories are rank-sorted by size and assigned so that slot s holds
same-rank categories on every core; slot widths W[s] (= max size in the rank
group) are baked into the single SPMD program. Slots are laid out so adjacent
pairs form blocks >= 256 columns wide (fp32r matmuls below 256 output columns
run at 1/4 rate).

neg_T uses relu instead of softplus: S entries have std ~89, so the
log1p(exp(-|S|)) correction to the 100+-term *mean* inside a log contributes
~2e-5 relative error (measured) vs the 2e-2 budget. This removes the entire
Abs/Exp/Ln/reduce elementwise tail over S that dominated the previous version.
The T term (diagonal) keeps the exact softplus path: its error enters the
output 1:1, and log(T + 1e-8) needs absolute accuracy when d is very negative
(exp + branch-free log1p with a series branch for e < 1e-4).

Per block (2 slots): DMA z,x -> fz = Wz^T z + bz x mrow (rank-1 keeps padded
columns exactly zero) -> h1 = relu(W1^T x + b1) (DVE, into a 51-row tile whose
last row is ones) -> u = (W2 Ws[g] | b2 Ws[g])^T h1_aug (bias folded via the
ones row; rhs widened to 256 in-block for full-rate fp32r) -> bf16 casts of
fz/u on ACT straight from PSUM -> per 128-chunk: S = u16^T fz16 (bf16),
d-column via (u*fz summed by a ones-matmul), relu row-sums accumulated from
PSUM alternating ACT/DVE.
"""

import sys

for _p in ("/opt/trn_rl_repo", "/root/.axon_site/_ro/trn_rl_repo"):
    if _p not in sys.path:
        sys.path.append(_p)

import numpy as np

import concourse.bacc as bacc
import concourse.tile as tile
from concourse import mybir as mb
from concourse.bass_utils import run_bass_kernel_spmd

# ---------------------------------------------------------------- constants
N, IN, Z, C, H = 8192, 512, 128, 64, 50
NCORES = 8
G = C // NCORES          # category slots per core
KX = IN // 128           # k-tiles for x
EPS = 1e-8
N_WARM = 8

HA = 64                  # partition row holding the ones for the folded u bias

F = mb.ActivationFunctionType
OP = mb.AluOpType
FP32 = mb.dt.float32
FP32R = mb.dt.float32r
BF16 = mb.dt.bfloat16

_PROGRAMS = {}


class Layout:
    """Slot/chunk/block geometry baked into the program (shared by cores)."""

    def __init__(self, widths):
        assert len(widths) == G
        self.W = list(widths)
        self.OFF = np.concatenate([[0], np.cumsum(self.W)]).astype(int)
        self.R = int(self.OFF[-1])
        # chunks: (slot, coff, cw, ci)
        self.chunks = []
        for s, w in enumerate(self.W):
            for coff in range(0, w, 128):
                self.chunks.append((s, coff, min(128, w - coff), len(self.chunks)))
        self.NCHUNK = len(self.chunks)
        self.blocks = [(s, min(s + 2, G)) for s in range(0, G, 2)]
        # packB column layout
        self.PK_W1 = (0, KX * H)
        self.PK_WZ = (KX * H, KX * H + Z)
        o = KX * H + Z
        self.PK_PINV = (o, o + self.NCHUNK)
        o += self.NCHUNK
        self.PK_B1 = (o, o + 1)
        self.PW = o + 1
        self.PK_BZ = (0, Z)
        self.PK_MROW = (Z, Z + self.R)
        self.PW1 = Z + self.R

    def key(self):
        return tuple(self.W)


def _build_program(L: Layout):
    nc = bacc.Bacc("TRN2", target_bir_lowering=False, debug=False)

    R, NC_ = L.R, L.NCHUNK
    d_xgT = nc.dram_tensor("xgT", [IN, R], FP32, kind="ExternalInput").ap()
    d_zgT = nc.dram_tensor("zgT", [Z, R], FP32, kind="ExternalInput").ap()
    d_packB = nc.dram_tensor("packB", [128, L.PW], FP32, kind="ExternalInput").ap()
    d_pack1 = nc.dram_tensor("pack1", [1, L.PW1], FP32, kind="ExternalInput").ap()
    d_w2s = nc.dram_tensor("w2s", [HA + 1, G * Z], FP32, kind="ExternalInput").ap()
    d_yout = nc.dram_tensor("yout", [NC_ * 128], FP32, kind="ExternalOutput").ap()

    with tile.TileContext(nc) as tc:
        with (
            tc.tile_pool(name="const", bufs=1) as const,
            tc.tile_pool(name="junk", bufs=3) as junkp,
            tc.tile_pool(name="blk", bufs=2) as blkp,
            tc.tile_pool(name="psum_z", bufs=2, space="PSUM") as psum_z,
            tc.tile_pool(name="psum_h", bufs=2, space="PSUM") as psum_h,
            tc.tile_pool(name="psum_u", bufs=1, space="PSUM") as psum_u,
            tc.tile_pool(name="psum_s", bufs=1, space="PSUM") as psum_s,
            tc.tile_pool(name="psum_d", bufs=1, space="PSUM") as psum_d,
        ):
            # ---- constants
            s_ones = const.tile([128, 1], FP32)
            nc.vector.memset(s_ones[:], 1.0)
            s_eps = const.tile([128, 1], FP32)
            nc.vector.memset(s_eps[:], EPS)
            # Pre-load the one ACT table set (id 6: natural_log_exp_and_others)
            # containing every function used here (Copy/Relu/Abs/Exp/Ln); the
            # automatic chooser would thrash tables mid-kernel.
            nc.scalar.add_instruction(
                mb.InstLoadActFuncSet(
                    name=nc.get_next_instruction_name(),
                    ins=[],
                    outs=[],
                    act_func_set_id=6,
                )
            )
            s_warmact = const.tile([128, 1], FP32)
            nc.scalar.activation(out=s_warmact[:], in_=s_ones[:], func=F.Abs)

            s_packB = const.tile([128, L.PW], FP32R)
            nc.sync.dma_start(out=s_packB[:], in_=d_packB.bitcast(FP32R)[:])
            s_pack1 = const.tile([1, L.PW1], FP32R)
            nc.sync.dma_start(out=s_pack1[:], in_=d_pack1.bitcast(FP32R)[:])

            def pk(lo_hi, rows=128, cast=None):
                ap = s_packB[0:rows, lo_hi[0] : lo_hi[1]]
                return ap.bitcast(cast) if cast else ap

            s_w1 = pk(L.PK_W1).rearrange("p (k h) -> p k h", k=KX)
            s_wz = pk(L.PK_WZ)
            s_pinv = pk(L.PK_PINV, cast=FP32)
            s_b1 = pk(L.PK_B1, rows=H, cast=FP32)
            s_bz = s_pack1[0:1, L.PK_BZ[0] : L.PK_BZ[1]]
            s_mrow = s_pack1[0:1, L.PK_MROW[0] : L.PK_MROW[1]]

            # PE warm-up to start the p-state ramp while DMA runs
            pwarm = psum_z.tile([1, 64], FP32, tag="pz")
            s_wrhs = const.tile([128, 64], FP32)
            nc.vector.memset(s_wrhs[:], 0.0)
            for _ in range(N_WARM):
                nc.tensor.matmul(
                    pwarm[:], lhsT=s_ones[:], rhs=s_wrhs[:], start=True, stop=True
                )

            # ---- persistent tiles
            s_zgT = const.tile([128, R], FP32R)
            s_xgT = const.tile([128, KX, R], FP32R)
            s_h1T = const.tile([HA + 1, R], FP32R)
            s_fz16 = const.tile([128, R], BF16)
            s_rel = const.tile([128, NC_], FP32)
            s_q2 = const.tile([128, NC_], FP32)

            s_w2s = const.tile([HA + 1, G * Z], FP32R)
            nc.sync.dma_start(out=s_w2s[:], in_=d_w2s.bitcast(FP32R)[:])

            # ones row (partition HA) for the folded u bias: u = W2s_aug^T
            # [h1; ...; 1]. Rows H..HA are zeroed (engine partition starts
            # must be multiples of 32; rows 32..50 are overwritten by h1).
            nc.vector.memset(s_h1T.bitcast(FP32)[32:HA, :], 0.0)
            nc.vector.memset(s_h1T.bitcast(FP32)[HA : HA + 1, :], 1.0)

            x_view = d_xgT.bitcast(FP32R).rearrange("(k p) n -> p k n", p=128)
            pd = psum_d.tile([128, NC_], FP32)
            # chunks narrower than 128 leave tail partitions untouched; zero
            # them once so the full-width T/neg_T chains stay defined
            nc.vector.memset(s_rel[:], 0.0)
            nc.vector.memset(pd[:], 0.0)

            for bi, (s0, s1) in enumerate(L.blocks):
                boff = int(L.OFF[s0])
                bw = int(L.OFF[s1] - L.OFF[s0])
                ns = slice(boff, boff + bw)
                nc.sync.dma_start(out=s_zgT[:, ns], in_=d_zgT.bitcast(FP32R)[:, ns])
                for k in range(KX):
                    nc.sync.dma_start(out=s_xgT[:, k, ns], in_=x_view[:, k, ns])

                # fz = Wz^T z + bz x mrow (rank-1 keeps padded columns zero)
                pz = psum_z.tile([128, bw], FP32, tag="pz")
                nc.tensor.matmul(
                    pz[:], lhsT=s_wz, rhs=s_zgT[:, ns], start=True, stop=False
                )
                nc.tensor.matmul(
                    pz[:], lhsT=s_bz, rhs=s_mrow[:, ns], start=False, stop=True
                )
                nc.scalar.activation(out=s_fz16[:, ns], in_=pz[:], func=F.Copy)

                # h1 = relu(W1^T x + b1)
                ph = psum_h.tile([H, bw], FP32, tag="ph")
                for k in range(KX):
                    nc.tensor.matmul(
                        ph[:],
                        lhsT=s_w1[:, k, :],
                        rhs=s_xgT[:, k, ns],
                        start=(k == 0),
                        stop=(k == KX - 1),
                    )
                nc.vector.tensor_scalar(
                    out=s_h1T.bitcast(FP32)[0:H, ns], in0=ph[:], scalar1=s_b1,
                    scalar2=0.0, op0=OP.add, op1=OP.max,
                )

                # u per slot; rhs widened to 256 (in-block) for 1-cyc fp32r
                nsb = s1 - s0
                pu = psum_u.tile([128, nsb, 256], FP32, tag="pu")
                ush = []
                for j, s in enumerate(range(s0, s1)):
                    rhs_off = min(int(L.OFF[s]), boff + bw - 256)
                    ush.append(int(L.OFF[s]) - rhs_off)
                    nc.tensor.matmul(
                        pu[:, j, :],
                        lhsT=s_w2s[:, s * Z : (s + 1) * Z],
                        rhs=s_h1T[:, rhs_off : rhs_off + 256],
                        start=True,
                        stop=True,
                    )
                s_u16b = blkp.tile([128, nsb, 256], BF16, tag="u16")
                nc.scalar.activation(out=s_u16b[:], in_=pu[:], func=F.Copy)

                bchunks = [ch for ch in L.chunks if s0 <= ch[0] < s1]
                pS = psum_s.tile([128, len(bchunks), 256], FP32, tag="ps")
                s_prodb = blkp.tile([128, nsb, 256], FP32, tag="prod")
                alt = 0
                for j, s in enumerate(range(s0, s1)):
                    w = L.W[s]
                    soff = int(L.OFF[s])
                    # prod = u * fz, both straight from PSUM
                    nc.vector.tensor_mul(
                        s_prodb[:, j, ush[j] : ush[j] + w],
                        pu[:, j, ush[j] : ush[j] + w],
                        pz[:, soff - boff : soff - boff + w],
                    )
                    for (cs, coff, cw, ci) in bchunks:
                        if cs != s:
                            continue
                        nc.tensor.matmul(
                            pd[0:cw, ci : ci + 1],
                            lhsT=s_prodb[:, j, ush[j] + coff : ush[j] + coff + cw],
                            rhs=s_ones[:],
                            start=True,
                            stop=True,
                        )
                        ck = ci - bchunks[0][3]
                        nc.tensor.matmul(
                            pS[0:cw, ck, 0:w],
                            lhsT=s_u16b[:, j, ush[j] + coff : ush[j] + coff + cw],
                            rhs=s_fz16[:, soff : soff + w],
                            start=True,
                            stop=True,
                        )
                        jk = junkp.tile([128, 256], FP32, tag="junk")
                        if alt == 0:
                            nc.vector.tensor_scalar(
                                out=jk[0:cw, 0:w], in0=pS[0:cw, ck, 0:w],
                                scalar1=0.0, scalar2=None, op0=OP.max, op1=OP.add,
                                accum_out=s_rel[0:cw, ci : ci + 1],
                            )
                        else:
                            nc.scalar.activation(
                                out=jk[0:cw, 0:w], in_=pS[0:cw, ck, 0:w],
                                func=F.Relu,
                                accum_out=s_rel[0:cw, ci : ci + 1],
                            )
                        alt ^= 1
                # neg_T numerator for this block's chunks
                cs0, cs1 = bchunks[0][3], bchunks[-1][3] + 1
                nc.vector.tensor_mul(
                    s_q2[:, cs0:cs1], s_rel[:, cs0:cs1], s_pinv[:, cs0:cs1]
                )

            # ---- T-term: softplus(d) = relu(d) + log1p(exp(-|d|)); series
            # branch for absolute accuracy near the 1e-8 floor
            sm = const
            s_ad = sm.tile([128, NC_], FP32)
            nc.scalar.activation(out=s_ad[:], in_=pd[:], func=F.Abs)
            s_ed = sm.tile([128, NC_], FP32)
            nc.scalar.activation(out=s_ed[:], in_=s_ad[:], func=F.Exp, scale=-1.0)
            s_lT = sm.tile([128, NC_], FP32)
            nc.scalar.activation(out=s_lT[:], in_=s_ed[:], func=F.Ln, bias=1.0)
            s_t1 = sm.tile([128, NC_], FP32)
            nc.vector.tensor_scalar(
                out=s_t1[:], in0=s_ed[:], scalar1=-0.5, scalar2=1.0,
                op0=OP.mult, op1=OP.add,
            )  # 1 - e/2
            s_ssm = sm.tile([128, NC_], FP32)
            nc.vector.tensor_mul(s_ssm[:], s_ed[:], s_t1[:])  # e*(1 - e/2)
            s_mk = sm.tile([128, NC_], mb.dt.int8)
            nc.vector.tensor_scalar(
                out=s_mk[:], in0=s_ed[:], scalar1=1e-4, scalar2=None, op0=OP.is_lt
            )
            s_l1p = sm.tile([128, NC_], FP32)
            nc.vector.select(s_l1p[:], s_mk[:], s_ssm[:], s_lT[:])
            s_T0 = sm.tile([128, NC_], FP32)
            nc.vector.scalar_tensor_tensor(
                out=s_T0[:], in0=pd[:], scalar=0.0, in1=s_l1p[:],
                op0=OP.max, op1=OP.add,
            )  # relu(d) + log1p(exp(-|d|))
            s_logT = sm.tile([128, NC_], FP32)
            nc.scalar.activation(out=s_logT[:], in_=s_T0[:], func=F.Ln, bias=s_eps[:])

            s_logn = sm.tile([128, NC_], FP32)
            nc.scalar.activation(out=s_logn[:], in_=s_q2[:], func=F.Ln, bias=s_eps[:])
            s_y = sm.tile([128, NC_], FP32)
            nc.vector.tensor_sub(s_y[:], s_logT[:], s_logn[:])

            nc.sync.dma_start(
                out=d_yout.rearrange("(c p) -> p c", p=128), in_=s_y[:]
            )

    nc.compile()
    return nc


def get_program(L: Layout):
    k = L.key()
    if k not in _PROGRAMS:
        _PROGRAMS[k] = _build_program(L)
    return _PROGRAMS[k]


# ---------------------------------------------------------------- host side
def _assign(cf):
    """Rank-sort categories, assign rank group s to slot position POS[s]."""
    sizes = np.array([(cf == k).sum() for k in range(C)])
    order = np.argsort(-sizes, kind="stable")
    # slot positions interleave large/small rank groups so adjacent pairs
    # (the matmul blocks) are >= 256 wide
    pos_of_group = [0, 2, 4, 6, 7, 5, 3, 1]  # rank group g -> position
    widths = [0] * G
    catmap = [[0] * G for _ in range(NCORES)]  # catmap[core][pos]
    nmap = [[0] * G for _ in range(NCORES)]
    for g in range(G):
        grp = order[8 * g : 8 * g + 8]
        p = pos_of_group[g]
        widths[p] = int(sizes[grp[0]])
        for core in range(NCORES):
            catmap[core][p] = int(grp[core])
            nmap[core][p] = int(sizes[grp[core]])
    return widths, catmap, nmap


def _prep_core_inputs(L, x, z, Ws, W1, b1, W2, b2, Wz, bz, idx_lists, catmap_c, nmap_c):
    xgT = np.zeros((IN, L.R), np.float32)
    zgT = np.zeros((Z, L.R), np.float32)
    pack1 = np.zeros((1, L.PW1), np.float32)
    pinv = np.ones((128, L.NCHUNK), np.float32)
    for s in range(G):
        idx = idx_lists[catmap_c[s]]
        n = nmap_c[s]
        lo = int(L.OFF[s])
        if n:
            xgT[:, lo : lo + n] = x[idx].T
            zgT[:, lo : lo + n] = z[idx].T
            pack1[0, L.PK_MROW[0] + lo : L.PK_MROW[0] + lo + n] = 1.0
    for (s, coff, cw, ci) in L.chunks:
        pinv[:, ci] = 1.0 / max(nmap_c[s], 1)
    pack1[0, L.PK_BZ[0] : L.PK_BZ[1]] = bz

    packB = np.zeros((128, L.PW), np.float32)
    packB[:, L.PK_W1[0] : L.PK_W1[1]] = (
        W1.reshape(KX, 128, H).transpose(1, 0, 2).reshape(128, KX * H)
    )
    packB[:, L.PK_WZ[0] : L.PK_WZ[1]] = Wz
    packB[:, L.PK_PINV[0] : L.PK_PINV[1]] = pinv
    packB[:H, L.PK_B1[0]] = b1
    # fold the second MLP layer and its bias into each slot's bilinear weight:
    # u = [h1; 1]^T [W2 Ws[g]; b2 Ws[g]]
    w2s = np.zeros((HA + 1, G * Z), np.float32)
    for s in range(G):
        Wsg = Ws[catmap_c[s]].astype(np.float64)
        w2s[:H, s * Z : (s + 1) * Z] = (W2.astype(np.float64) @ Wsg).astype(np.float32)
        w2s[HA, s * Z : (s + 1) * Z] = (b2.astype(np.float64) @ Wsg).astype(np.float32)
    return {"xgT": xgT, "zgT": zgT, "packB": packB, "pack1": pack1, "w2s": w2s}


def _unpack_core_output(L, y, idx_lists, catmap_c, nmap_c, out):
    for (s, coff, cw, ci) in L.chunks:
        n = nmap_c[s]
        take = min(cw, n - coff)
        if take > 0:
            idx = idx_lists[catmap_c[s]][coff : coff + take]
            out[idx] = y[ci * 128 : ci * 128 + take]


def _numpy_fallback(x, c, z, W1, b1, W2, b2, Wz, bz, Ws):
    x64 = x.astype(np.float64)
    fx = np.maximum(x64 @ W1.astype(np.float64) + b1, 0.0) @ W2.astype(
        np.float64
    ) + b2
    fz = z.astype(np.float64) @ Wz.astype(np.float64) + bz
    u = np.einsum("nd,nde->ne", fx, Ws.astype(np.float64)[c])

    def sp(v):
        return np.log1p(np.exp(-np.abs(v))) + np.maximum(v, 0.0)

    T = sp(np.einsum("ne,ne->n", u, fz))
    out = np.empty(N, np.float64)
    for k in range(C):
        idx = np.where(c == k)[0]
        if len(idx) == 0:
            continue
        Sk = sp(u[idx] @ fz[idx].T)
        out[idx] = np.log(T[idx] + EPS) - np.log(Sk.mean(axis=1) + EPS)
    return out.astype(np.float32)


def kernel(x, c, z, W1, b1, W2, b2, Wz, bz, Ws):
    x = np.ascontiguousarray(np.asarray(x), dtype=np.float32)
    z = np.ascontiguousarray(np.asarray(z), dtype=np.float32)
    W1 = np.ascontiguousarray(np.asarray(W1), dtype=np.float32)
    b1 = np.ascontiguousarray(np.asarray(b1), dtype=np.float32)
    W2 = np.ascontiguousarray(np.asarray(W2), dtype=np.float32)
    b2 = np.ascontiguousarray(np.asarray(b2), dtype=np.float32)
    Wz = np.ascontiguousarray(np.asarray(Wz), dtype=np.float32)
    bz = np.ascontiguousarray(np.asarray(bz), dtype=np.float32)
    Ws = np.ascontiguousarray(np.asarray(Ws), dtype=np.float32)
    cf = np.asarray(c).reshape(-1).astype(np.int64)

    idx_lists = [np.where(cf == k)[0] for k in range(C)]
    sizes = [len(i) for i in idx_lists]
    if max(sizes) > 256 or min(sizes) == 0 or len(cf) != N:
        return _numpy_fallback(x, cf, z, W1, b1, W2, b2, Wz, bz, Ws)

    widths, catmap, nmap = _assign(cf)
    L = Layout(widths)

    in_maps = [
        _prep_core_inputs(
            L, x, z, Ws, W1, b1, W2, b2, Wz, bz, idx_lists, catmap[core], nmap[core]
        )
        for core in range(NCORES)
    ]

    nc = get_program(L)
    res = run_bass_kernel_spmd(nc, in_maps, core_ids=list(range(NCORES)))

    out = np.empty(N, np.float32)
    for core in range(NCORES):
        _unpack_core_output(
            L, res.results[core]["yout"], idx_lists, catmap[core], nmap[core], out
        )
    return out


# revision 13
# speedup vs baseline: 1.5460x; 1.1356x over previous
"""Trainium2 Bass kernel for the CPC contrastive loss problem.

Math (reference):
    fx = relu(x @ W1 + b1) @ W2 + b2          [N, Z]
    fz = z @ Wz + bz                          [N, Z]
    u[n] = fx[n] @ Ws[c[n]]                   [N, Z]
    T = softplus(<u, fz>_row)                 [N]
    neg_T[i] = mean_{j: c[j]==c[i]} softplus(<u[i], fz[j]>)
    out = log(T + eps) - log(neg_T + eps)

Structure: rows are grouped by category on the host; each of the 8 cores gets
8 categories, so the NxN S matrix reduces to per-category blocks (64x less
work). Categories are rank-sorted by size; slot s holds same-rank categories
on every core, so the slot widths W[s] (max size in the rank group) bake into
one SPMD program. Slot positions interleave large/small ranks so adjacent
pairs (the processing blocks) are >= 256 columns wide: fp32r matmuls below
256 output columns run at 1/4 rate.

Key optimizations vs a straight port:
  - x and z ship as bf16 hi+lo pairs (halves the dominant DMA volume); W1/Wz
    stay fp32r, so the product precision is unchanged (the hi+lo pair
    reconstructs the input to ~2^-17, and fp32r weight rounding dominates).
  - neg_T uses relu instead of softplus: S entries have std ~89, so the
    log1p(exp(-|S|)) correction inside a 100+-term mean inside a log is
    ~2e-5 relative (measured) vs the 2e-2 budget. This deletes the entire
    Abs/Exp/Ln/reduce elementwise tail over S.
  - One DMA per block (z and x halves stacked in one dram tensor): the cost
    model charges ~650ns of issue time per DMA, so few large transfers beat
    many small ones.
  - The device returns d = <u,fz> and q2 = mean_j relu(S) per row; the final
    log(softplus(d)+eps) - log(q2+eps) is O(N) scalar work done in float64
    on the host during unsharding (exact softplus, no LUT range issues).
  - u's bias is folded into the matmul via an all-ones row at partition HA
    of the h1 tile (engine partition starts must be multiples of 32).
  - PE work is software-pipelined one block behind the fz/h1 matmuls so the
    in-order PE queue never head-blocks waiting for the bf16 casts.
  - Elementwise work is spread over DVE/ACT/Pool; Pool reads PSUM fine.
"""

import sys

for _p in ("/opt/trn_rl_repo", "/root/.axon_site/_ro/trn_rl_repo"):
    if _p not in sys.path:
        sys.path.append(_p)

import numpy as np
import ml_dtypes

import concourse.bacc as bacc
import concourse.tile as tile
from concourse import mybir as mb
from concourse.bass_utils import run_bass_kernel_spmd

BF16NP = ml_dtypes.bfloat16

# ---------------------------------------------------------------- constants
N, IN, Z, C, H = 8192, 512, 128, 64, 50
NCORES = 8
G = C // NCORES          # category slots per core
KX = IN // 128           # k-tiles for x
KZ = 2 + 2 * KX          # bf16 row-groups in the xz tensor: zh zl xh*4 xl*4
EPS = 1e-8
N_WARM = 8
HA = 64                  # partition row holding the ones for the folded u bias

F = mb.ActivationFunctionType
OP = mb.AluOpType
FP32 = mb.dt.float32
FP32R = mb.dt.float32r
BF16 = mb.dt.bfloat16

_PROGRAMS = {}


class Layout:
    """Slot/chunk/block geometry baked into the program (shared by cores)."""

    def __init__(self, widths):
        assert len(widths) == G
        self.W = list(widths)
        self.OFF = np.concatenate([[0], np.cumsum(self.W)]).astype(int)
        self.R = int(self.OFF[-1])
        # chunks: (slot, coff, cw, ci)
        self.chunks = []
        for s, w in enumerate(self.W):
            for coff in range(0, w, 128):
                self.chunks.append((s, coff, min(128, w - coff), len(self.chunks)))
        self.NCHUNK = len(self.chunks)
        self.blocks = [(s, min(s + 2, G)) for s in range(0, G, 2)]
        # packA column layout
        self.PK_W1 = (0, KX * H)
        self.PK_WZ = (KX * H, KX * H + Z)
        o = KX * H + Z
        self.PK_PINV = (o, o + self.NCHUNK)
        o += self.NCHUNK
        self.PK_B1 = (o, o + 1)
        self.PW = o + 1
        self.PK_BZ = (0, Z)
        self.PK_MROW = (Z, Z + self.R)
        self.PW1 = Z + self.R

    def ok(self):
        return all(
            int(self.OFF[s1] - self.OFF[s0]) >= 256 for s0, s1 in self.blocks
        ) and max(self.W) <= 256

    def key(self):
        return tuple(self.W)


def _build_program(L: Layout):
    nc = bacc.Bacc("TRN2", target_bir_lowering=False, debug=False)

    R, NC_ = L.R, L.NCHUNK
    d_xz = nc.dram_tensor("xz", [KZ * 128, R], BF16, kind="ExternalInput").ap()
    d_packA = nc.dram_tensor("packA", [128, L.PW], FP32, kind="ExternalInput").ap()
    d_pack1 = nc.dram_tensor("pack1", [1, L.PW1], FP32, kind="ExternalInput").ap()
    d_w2s = nc.dram_tensor("w2s", [HA + 1, G * Z], FP32, kind="ExternalInput").ap()
    d_yout = nc.dram_tensor("yout", [128 * 2 * NC_], FP32, kind="ExternalOutput").ap()

    xz_view = d_xz.rearrange("(k p) n -> p k n", p=128)

    with tile.TileContext(nc) as tc:
        with (
            tc.tile_pool(name="const", bufs=1) as const,
            tc.tile_pool(name="junk", bufs=3) as junkp,
            tc.tile_pool(name="blk", bufs=2) as blkp,
            tc.tile_pool(name="psum_z", bufs=2, space="PSUM") as psum_z,
            tc.tile_pool(name="psum_h", bufs=2, space="PSUM") as psum_h,
            tc.tile_pool(name="psum_u", bufs=1, space="PSUM") as psum_u,
            tc.tile_pool(name="psum_s", bufs=1, space="PSUM") as psum_s,
            tc.tile_pool(name="psum_d", bufs=1, space="PSUM") as psum_d,
        ):
            # ---- constants
            s_ones = const.tile([128, 1], FP32)
            nc.vector.memset(s_ones[:], 1.0)
            # the one ACT table set (id 6) holding Copy/Relu used below
            nc.scalar.add_instruction(
                mb.InstLoadActFuncSet(
                    name=nc.get_next_instruction_name(),
                    ins=[],
                    outs=[],
                    act_func_set_id=6,
                )
            )
            s_warmact = const.tile([128, 1], FP32)
            nc.scalar.activation(out=s_warmact[:], in_=s_ones[:], func=F.Abs)

            # ---- persistent tiles
            s_xz = const.tile([128, KZ, R], BF16)
            s_h1T = const.tile([HA + 1, R], FP32R)
            s_fz16 = const.tile([128, R], BF16)
            s_out = const.tile([128, 2, NC_], FP32)  # [:,0,:] q2, [:,1,:] d
            s_packA = const.tile([128, L.PW], FP32R)
            s_pack1 = const.tile([1, L.PW1], FP32R)
            s_w2s = const.tile([HA + 1, G * Z], FP32R)

            # ---- all DMAs up front in issue order
            nc.sync.dma_start(out=s_packA[:], in_=d_packA.bitcast(FP32R)[:])
            nc.sync.dma_start(out=s_pack1[:], in_=d_pack1.bitcast(FP32R)[:])
            for bi, (s0, s1) in enumerate(L.blocks):
                ns = slice(int(L.OFF[s0]), int(L.OFF[s1]))
                nc.sync.dma_start(out=s_xz[:, :, ns], in_=xz_view[:, :, ns])
                if bi == 0:
                    nc.sync.dma_start(out=s_w2s[:], in_=d_w2s.bitcast(FP32R)[:])

            def pk(lo_hi, rows=128, cast=None):
                ap = s_packA[0:rows, lo_hi[0] : lo_hi[1]]
                return ap.bitcast(cast) if cast else ap

            s_w1 = pk(L.PK_W1).rearrange("p (k h) -> p k h", k=KX)
            s_wz = pk(L.PK_WZ)
            s_pinv = pk(L.PK_PINV, cast=FP32)
            s_b1 = pk(L.PK_B1, rows=H, cast=FP32)
            s_bz = s_pack1[0:1, L.PK_BZ[0] : L.PK_BZ[1]]
            s_mrow = s_pack1[0:1, L.PK_MROW[0] : L.PK_MROW[1]]

            # PE warm-up to start the p-state ramp while DMA runs
            pwarm = psum_z.tile([1, 64], FP32, tag="pz")
            s_wrhs = const.tile([128, 64], FP32)
            nc.vector.memset(s_wrhs[:], 0.0)
            for _ in range(N_WARM):
                nc.tensor.matmul(
                    pwarm[:], lhsT=s_ones[:], rhs=s_wrhs[:], start=True, stop=True
                )

            # ones row (partition HA) for the folded u bias: u = W2s_aug^T
            # [h1; ...; 1]. Rows H..HA zeroed (partition starts must be
            # multiples of 32; rows 32..H are overwritten by every h1 block).
            nc.vector.memset(s_h1T.bitcast(FP32)[32:HA, :], 0.0)
            nc.vector.memset(s_h1T.bitcast(FP32)[HA : HA + 1, :], 1.0)
            # chunks narrower than 128 leave tail partitions untouched
            nc.vector.memset(s_out[:], 0.0)
            pd = psum_d.tile([128, NC_], FP32)
            nc.vector.memset(pd[:], 0.0)

            state = {}

            def emit_uS(bi):
                """u matmuls, casts, prod/d, S and relu-accums for block bi."""
                s0, s1 = L.blocks[bi]
                boff = int(L.OFF[s0])
                bw = int(L.OFF[s1] - L.OFF[s0])
                pz = state[bi]["pz"]
                nsb = s1 - s0
                pu = psum_u.tile([128, nsb, 256], FP32, tag="pu")
                ush = []
                for j, s in enumerate(range(s0, s1)):
                    rhs_off = min(int(L.OFF[s]), boff + bw - 256)
                    ush.append(int(L.OFF[s]) - rhs_off)
                    nc.tensor.matmul(
                        pu[:, j, :],
                        lhsT=s_w2s[:, s * Z : (s + 1) * Z],
                        rhs=s_h1T[:, rhs_off : rhs_off + 256],
                        start=True,
                        stop=True,
                    )
                last = bi == len(L.blocks) - 1
                s_u16b = blkp.tile([128, nsb, 256], BF16, tag="u16")
                if not last:
                    nc.gpsimd.tensor_copy(s_u16b[:], pu[:])
                else:
                    for j in range(nsb):  # per-slot on ACT: shorter tail chain
                        nc.scalar.activation(
                            out=s_u16b[:, j, :], in_=pu[:, j, :], func=F.Copy
                        )

                bchunks = [ch for ch in L.chunks if s0 <= ch[0] < s1]
                pS = psum_s.tile([128, len(bchunks), 256], FP32, tag="ps")
                s_prodb = blkp.tile([128, nsb, 256], FP32, tag="prod")
                for j, s in enumerate(range(s0, s1)):
                    w = L.W[s]
                    soff = int(L.OFF[s])
                    nc.vector.tensor_mul(
                        s_prodb[:, j, ush[j] : ush[j] + w],
                        pu[:, j, ush[j] : ush[j] + w],
                        pz[:, soff - boff : soff - boff + w],
                    )
                    for (cs, coff, cw, ci) in bchunks:
                        if cs != s:
                            continue
                        nc.tensor.matmul(
                            pd[0:cw, ci : ci + 1],
                            lhsT=s_prodb[:, j, ush[j] + coff : ush[j] + coff + cw],
                            rhs=s_ones[:],
                            start=True,
                            stop=True,
                        )
                        ck = ci - bchunks[0][3]
                        nc.tensor.matmul(
                            pS[0:cw, ck, 0:w],
                            lhsT=s_u16b[:, j, ush[j] + coff : ush[j] + coff + cw],
                            rhs=s_fz16[:, soff : soff + w],
                            start=True,
                            stop=True,
                        )
                        # q2 = pinv * sum_j relu(S) straight from PSUM; the
                        # per-chunk scale folds the 1/n mean into the accum
                        jk = junkp.tile([128, 256], FP32, tag="junk")
                        pv = s_pinv[0:cw, ci : ci + 1]
                        acc = s_out[0:cw, 0, ci : ci + 1]
                        if ci % 2 == 0:
                            nc.vector.tensor_scalar(
                                out=jk[0:cw, 0:w], in0=pS[0:cw, ck, 0:w],
                                scalar1=pv, scalar2=0.0, op0=OP.mult, op1=OP.max,
                                accum_out=acc,
                            )
                        else:
                            nc.scalar.activation(
                                out=jk[0:cw, 0:w], in_=pS[0:cw, ck, 0:w],
                                func=F.Relu, scale=pv, accum_out=acc,
                            )

            for bi, (s0, s1) in enumerate(L.blocks):
                boff = int(L.OFF[s0])
                bw = int(L.OFF[s1] - L.OFF[s0])
                ns = slice(boff, boff + bw)

                # fz = Wz^T (zh + zl) + bz x mrow (rank-1 keeps pads zero)
                pz = psum_z.tile([128, bw], FP32, tag="pz")
                nc.tensor.matmul(
                    pz[:], lhsT=s_wz, rhs=s_xz[:, 0, ns], start=True, stop=False
                )
                nc.tensor.matmul(
                    pz[:], lhsT=s_wz, rhs=s_xz[:, 1, ns], start=False, stop=False
                )
                nc.tensor.matmul(
                    pz[:], lhsT=s_bz, rhs=s_mrow[:, ns], start=False, stop=True
                )
                nc.gpsimd.tensor_copy(s_fz16[:, ns], pz[:])
                state[bi] = {"pz": pz}

                # h1 = relu(W1^T (xh + xl) + b1)
                ph = psum_h.tile([H, bw], FP32, tag="ph")
                for kk in range(2 * KX):
                    nc.tensor.matmul(
                        ph[:],
                        lhsT=s_w1[:, kk % KX, :],
                        rhs=s_xz[:, 2 + kk, ns],
                        start=(kk == 0),
                        stop=(kk == 2 * KX - 1),
                    )
                nc.vector.tensor_scalar(
                    out=s_h1T.bitcast(FP32)[0:H, ns], in0=ph[:], scalar1=s_b1,
                    scalar2=0.0, op0=OP.add, op1=OP.max,
                )
                if bi > 0:
                    emit_uS(bi - 1)
            emit_uS(len(L.blocks) - 1)

            # d column straight out of PSUM into the output tile
            nc.vector.tensor_copy(s_out[:, 1, :], pd[:])
            nc.sync.dma_start(
                out=d_yout.rearrange("(p t c) -> p t c", p=128, t=2), in_=s_out[:]
            )

    nc.compile()
    return nc


def get_program(L: Layout):
    k = L.key()
    if k not in _PROGRAMS:
        _PROGRAMS[k] = _build_program(L)
    return _PROGRAMS[k]


# ---------------------------------------------------------------- host side
def _assign(cf):
    """Rank-sort categories; rank group g goes to slot position POS[g] so
    adjacent slot pairs (the matmul blocks) are >= 256 wide."""
    sizes = np.array([(cf == k).sum() for k in range(C)])
    order = np.argsort(-sizes, kind="stable")
    pos_of_group = [0, 2, 4, 6, 7, 5, 3, 1]
    widths = [0] * G
    catmap = [[0] * G for _ in range(NCORES)]
    nmap = [[0] * G for _ in range(NCORES)]
    for g in range(G):
        grp = order[8 * g : 8 * g + 8]
        p = pos_of_group[g]
        widths[p] = int(sizes[grp[0]])
        for core in range(NCORES):
            catmap[core][p] = int(grp[core])
            nmap[core][p] = int(sizes[grp[core]])
    return widths, catmap, nmap


def _hi_lo(a):
    hi = a.astype(BF16NP)
    lo = (a - hi.astype(np.float32)).astype(BF16NP)
    return hi, lo


def _prep_core_inputs(L, x, z, Ws, W1, b1, W2, b2, Wz, bz, idx_lists, catmap_c, nmap_c):
    xz = np.zeros((KZ * 128, L.R), BF16NP)
    pack1 = np.zeros((1, L.PW1), np.float32)
    pinv = np.ones((128, L.NCHUNK), np.float32)
    for s in range(G):
        idx = idx_lists[catmap_c[s]]
        n = nmap_c[s]
        lo = int(L.OFF[s])
        if n:
            zh, zl = _hi_lo(z[idx].T)
            xz[0:128, lo : lo + n] = zh
            xz[128:256, lo : lo + n] = zl
            xh, xl = _hi_lo(x[idx].T)
            xz[256 : 256 + IN, lo : lo + n] = xh
            xz[256 + IN :, lo : lo + n] = xl
            pack1[0, L.PK_MROW[0] + lo : L.PK_MROW[0] + lo + n] = 1.0
    for (s, coff, cw, ci) in L.chunks:
        pinv[:, ci] = 1.0 / max(nmap_c[s], 1)
    pack1[0, L.PK_BZ[0] : L.PK_BZ[1]] = bz

    packA = np.zeros((128, L.PW), np.float32)
    packA[:, L.PK_W1[0] : L.PK_W1[1]] = (
        W1.reshape(KX, 128, H).transpose(1, 0, 2).reshape(128, KX * H)
    )
    packA[:, L.PK_WZ[0] : L.PK_WZ[1]] = Wz
    packA[:, L.PK_PINV[0] : L.PK_PINV[1]] = pinv
    packA[:H, L.PK_B1[0]] = b1
    # fold the second MLP layer and its bias into each slot's bilinear weight
    w2s = np.zeros((HA + 1, G * Z), np.float32)
    for s in range(G):
        Wsg = Ws[catmap_c[s]].astype(np.float64)
        w2s[:H, s * Z : (s + 1) * Z] = (W2.astype(np.float64) @ Wsg).astype(np.float32)
        w2s[HA, s * Z : (s + 1) * Z] = (b2.astype(np.float64) @ Wsg).astype(np.float32)
    return {"xz": xz, "packA": packA, "pack1": pack1, "w2s": w2s}


def _unpack_core_output(L, y, idx_lists, catmap_c, nmap_c, out):
    """y flat [(p t c)] -> rows; final log(softplus(d)+eps)-log(q2+eps) in
    float64 on the host (O(N) unshard-time scalar work)."""
    y = np.asarray(y).reshape(128, 2, L.NCHUNK).astype(np.float64)
    q2 = y[:, 0, :]
    d = y[:, 1, :]
    T = np.log1p(np.exp(-np.abs(d))) + np.maximum(d, 0.0)
    vals = np.log(T + EPS) - np.log(q2 + EPS)
    for (s, coff, cw, ci) in L.chunks:
        n = nmap_c[s]
        take = min(cw, n - coff)
        if take > 0:
            idx = idx_lists[catmap_c[s]][coff : coff + take]
            out[idx] = vals[0:take, ci]


def _numpy_fallback(x, c, z, W1, b1, W2, b2, Wz, bz, Ws):
    x64 = x.astype(np.float64)
    fx = np.maximum(x64 @ W1.astype(np.float64) + b1, 0.0) @ W2.astype(
        np.float64
    ) + b2
    fz = z.astype(np.float64) @ Wz.astype(np.float64) + bz
    u = np.einsum("nd,nde->ne", fx, Ws.astype(np.float64)[c])

    def sp(v):
        return np.log1p(np.exp(-np.abs(v))) + np.maximum(v, 0.0)

    T = sp(np.einsum("ne,ne->n", u, fz))
    out = np.empty(N, np.float64)
    for k in range(C):
        idx = np.where(c == k)[0]
        if len(idx) == 0:
            continue
        Sk = sp(u[idx] @ fz[idx].T)
        out[idx] = np.log(T[idx] + EPS) - np.log(Sk.mean(axis=1) + EPS)
    return out.astype(np.float32)


def kernel(x, c, z, W1, b1, W2, b2, Wz, bz, Ws):
    x = np.ascontiguousarray(np.asarray(x), dtype=np.float32)
    z = np.ascontiguousarray(np.asarray(z), dtype=np.float32)
    W1 = np.ascontiguousarray(np.asarray(W1), dtype=np.float32)
    b1 = np.ascontiguousarray(np.asarray(b1), dtype=np.float32)
    W2 = np.ascontiguousarray(np.asarray(W2), dtype=np.float32)
    b2 = np.ascontiguousarray(np.asarray(b2), dtype=np.float32)
    Wz = np.ascontiguousarray(np.asarray(Wz), dtype=np.float32)
    bz = np.ascontiguousarray(np.asarray(bz), dtype=np.float32)
    Ws = np.ascontiguousarray(np.asarray(Ws), dtype=np.float32)
    cf = np.asarray(c).reshape(-1).astype(np.int64)

    idx_lists = [np.where(cf == k)[0] for k in range(C)]
    sizes = [len(i) for i in idx_lists]
    if max(sizes) > 256 or min(sizes) == 0 or len(cf) != N:
        return _numpy_fallback(x, cf, z, W1, b1, W2, b2, Wz, bz, Ws)

    widths, catmap, nmap = _assign(cf)
    L = Layout(widths)
    if not L.ok():
        return _numpy_fallback(x, cf, z, W1, b1, W2, b2, Wz, bz, Ws)

    in_maps = [
        _prep_core_inputs(
            L, x, z, Ws, W1, b1, W2, b2, Wz, bz, idx_lists, catmap[core], nmap[core]
        )
        for core in range(NCORES)
    ]

    nc = get_program(L)
    res = run_bass_kernel_spmd(nc, in_maps, core_ids=list(range(NCORES)))

    out = np.empty(N, np.float32)
    for core in range(NCORES):
        _unpack_core_output(
            L, res.results[core]["yout"], idx_lists, catmap[core], nmap[core], out
        )
    return out
